# revision 13
# baseline (speedup 1.0000x reference)
"""DiscreteHMM log-likelihood on 8 Trainium2 NeuronCores.

Math: the reference forward algorithm in log space,
    alpha_{t+1,j} = logsumexp_i(alpha_{t,i} + lA[i,j]) + lB[j, o_{t+1}]
is computed here in *probability* space (classic scaled forward algorithm):
    p_{t+1} = (p_t @ A) * E_{t+1},   A = softmax(log_A, rows), E_t = 1024*B[:, o_t]
The transition preserves total mass (A rows sum to 1) and the emission
multiply scales it by ~1/1024 on average (column means of a softmax row-
normalized 512x1024 table), so with the constant 1024 rescale folded into E
the running mass drifts only a few nats over all 512 steps (measured
[-4.1, +3.5] for these inputs) -- no per-step renormalization is needed.
Final per-sequence loglik = ln(sum_j p_T) - T*ln(1024).

Sharding: data-parallel over batch -- 8 sequences per core, parameters
replicated; per-sequence logliks are summed on host (64 adds).

Device layout (states-major): p is a (512 states x 8 batch) column block,
packed as ONE SBUF tile of (128, 32) bf16 -- column block m holds state
chunk j in [128m, 128m+128). Each step: 16 matmuls
psum[:, 8m:8m+8] += A[128k:,128m:].T @ p[:, 8k:8k+8] (A chunks stationary
128x128 bf16 weights, batch the 8-wide moving operand), then ONE DVE
multiply with the pre-gathered emission tile (128, 32) -> next p.
Emissions are gathered on host into a per-core stream with matching
(p, t, m, b) layout and double-buffered into SBUF in 64-step blocks.
"""

import numpy as np
import ml_dtypes
from contextlib import ExitStack

import concourse.bass as bass
import concourse.bacc as bacc
import concourse.mybir as mybir
import concourse.tile as tile
from concourse.bass_utils import run_bass_kernel_spmd

S = 512          # states
O = 1024         # observation symbols
B = 64           # batch
T = 512          # timesteps
NCORES = 8
BSH = B // NCORES          # sequences per core
P = 128                    # partition size
KC = S // P                # 4 state chunks
W = KC * BSH               # 32: packed free width of the p tile
TBLK = 64                  # timesteps per emission DMA block
NBLK = T // TBLK

F32 = mybir.dt.float32
BF16 = mybir.dt.bfloat16
_BF16_NP = ml_dtypes.bfloat16

_cached_nc = None


def _build_nc() -> bass.Bass:
    nc = bacc.Bacc()
    a_d = nc.dram_tensor("a_mat", (S, S), BF16, kind="ExternalInput")
    pi_d = nc.dram_tensor("pi_vec", (P, KC), F32, kind="ExternalInput")
    e_d = nc.dram_tensor("e_str", (NBLK, P, TBLK * W), F32, kind="ExternalInput")
    out_d = nc.dram_tensor("out_ll", (1, BSH), F32, kind="ExternalOutput")

    with ExitStack() as ctx:
        tc = ctx.enter_context(tile.TileContext(nc))
        const = ctx.enter_context(tc.tile_pool(name="const", bufs=1))
        epool = ctx.enter_context(tc.tile_pool(name="epool", bufs=2))
        ppool = ctx.enter_context(tc.tile_pool(name="ppool", bufs=5))
        pspool = ctx.enter_context(tc.tile_pool(name="psum", bufs=2, space="PSUM"))

        # prologue DMAs: one per A row-chunk (ordered by first use), one for
        # pi, and block-0 emissions as 4 quarter tiles so all transfers run
        # on parallel HWDGE queues.
        pi_t = const.tile([P, KC], F32, name="pi", tag="pi")
        nc.sync.dma_start(pi_t[:], pi_d[:, :])
        # block-0 emissions in uneven slices (first slice small so the scan
        # starts early); A chunks ordered by first use; late e slices issued
        # last so no early consumer shares a DMA-queue sem with them.
        E0SPLIT = (8, 8, 16, 32)
        E0OFF = (0, 8, 16, 32)
        e0q = []
        t_off = 0
        for i, n in enumerate(E0SPLIT):
            e0q.append(const.tile([P, n * W], F32, name=f"e0q{i}", tag=f"e0q{i}"))
        nc.sync.dma_start(e0q[0][:], e_d[0][:, 0:E0SPLIT[0] * W])
        a_t = {}
        for k in (2, 3, 0, 1):
            a_t[k] = const.tile([P, S], BF16, name=f"a{k}", tag=f"a{k}")
            nc.sync.dma_start(a_t[k][:], a_d[k * P:(k + 1) * P, :])
        for i in (1, 2, 3):
            nc.sync.dma_start(e0q[i][:],
                              e_d[0][:, E0OFF[i] * W:(E0OFF[i] + E0SPLIT[i]) * W])
        ones_t = const.tile([P, 1], BF16, name="ones", tag="ones")
        nc.vector.memset(ones_t[:], 1.0)

        def load_eblk(blk):
            et = epool.tile([P, TBLK * W], F32, name="eb", tag="eb")
            nc.sync.dma_start(et[:], e_d[blk])
            return et

        eb = None
        # p is held as two packed half tiles: pA = chunks {0,1}, pB = {2,3};
        # 3D (P, 2, BSH) so the DVE multiply covers both chunks in one op.
        pA = ppool.tile([P, 2, BSH], BF16, name="pA", tag="pA")
        pB = ppool.tile([P, 2, BSH], BF16, name="pB", tag="pB")
        for m in range(KC):
            dst = pA if m < 2 else pB
            nc.vector.tensor_scalar_mul(dst[:, m % 2, :],
                                        e0q[0][:, m * BSH:(m + 1) * BSH],
                                        pi_t[:, m:m + 1])

        def p_slice(k):
            src = pA if k < 2 else pB
            return src[:, k % 2, :]

        def e_slice(src_t, tt, half):
            ap = src_t[:, tt * W + half * 2 * BSH: tt * W + (half + 1) * 2 * BSH]
            return ap.rearrange("p (x b) -> p x b", b=BSH)

        # Matmul slot order + paired DVE multiplies chosen by simulating the
        # steady-state latency loop (MM drain -> sem -> DVE -> sem -> MM):
        # pair {2,3} completes early and feeds the first DVE op; its chunks
        # are consumed late in the next step. Each pair shares ONE psum bank
        # (contiguous (P,16) f32 so the DVE reads pipeline); the two groups
        # within a bank never interleave (in-bank group interleave is a HW
        # crash), while cross-bank interleaving is free.
        SLOTS = [(2, 2), (1, 3), (2, 3), (1, 2), (2, 0), (1, 0), (1, 1), (2, 1),
                 (3, 1), (3, 2), (3, 3), (0, 2), (3, 0), (0, 1), (0, 3), (0, 0)]
        for blk in range(NBLK):
            if blk > 0:
                eb = load_eblk(blk)
            for tt in range(1 if blk == 0 else 0, TBLK):
                psA = pspool.tile([P, 2 * BSH], F32, name="psA", tag="psA")
                psB = pspool.tile([P, 2 * BSH], F32, name="psB", tag="psB")
                done = [0] * KC
                for (m, k) in SLOTS:
                    dst = psA if m < 2 else psB
                    done[m] += 1
                    nc.tensor.matmul(dst[:, (m % 2) * BSH:(m % 2 + 1) * BSH],
                                     a_t[k][:, m * P:(m + 1) * P], p_slice(k),
                                     start=(done[m] == 1), stop=(done[m] == KC),
                                     skip_group_check=True)
                if blk == 0:
                    qi = 0 if tt < 8 else (1 if tt < 16 else (2 if tt < 32 else 3))
                    esrc, ett = e0q[qi], tt - E0OFF[qi]
                else:
                    esrc, ett = eb, tt
                pB = ppool.tile([P, 2, BSH], BF16, name="pB", tag="pB")
                nc.vector.tensor_mul(pB[:], psB[:].rearrange("p (x b) -> p x b", b=BSH),
                                     e_slice(esrc, ett, 1))
                pA = ppool.tile([P, 2, BSH], BF16, name="pA", tag="pA")
                nc.vector.tensor_mul(pA[:], psA[:].rearrange("p (x b) -> p x b", b=BSH),
                                     e_slice(esrc, ett, 0))

        msum = pspool.tile([1, BSH], F32, name="msum", tag="psA")
        for k in range(KC):
            nc.tensor.matmul(msum[:], ones_t[:], p_slice(k),
                             start=(k == 0), stop=(k == KC - 1))
        lls = const.tile([1, BSH], F32, name="ll", tag="ll")
        nc.scalar.activation(lls[:], msum[:], mybir.ActivationFunctionType.Ln)
        nc.sync.dma_start(out_d[:, :], lls[:])
    nc.finalize()
    return nc


def _softmax(x, axis):
    x = x - x.max(axis=axis, keepdims=True)
    e = np.exp(x)
    return e / e.sum(axis=axis, keepdims=True)


def kernel(observations, log_pi, log_A, log_B):
    global _cached_nc
    obs = np.asarray(observations)
    A = _softmax(np.asarray(log_A, dtype=np.float64), 1)
    Bp = _softmax(np.asarray(log_B, dtype=np.float64), 1).astype(np.float32)
    pi = _softmax(np.asarray(log_pi, dtype=np.float64), 0).astype(np.float32)

    a_bf = A.astype(_BF16_NP)
    pi_in = np.ascontiguousarray(pi.reshape(KC, P).T)
    # X[j, b, t] = 1024 * B[j, o_{b,t}]
    X = (np.float32(O) * Bp[:, obs]).astype(np.float32)

    in_maps = []
    for c in range(NCORES):
        xc = X[:, c * BSH:(c + 1) * BSH, :]                    # (S, BSH, T)
        ec = xc.reshape(KC, P, BSH, NBLK, TBLK)                # (m, p, b, blk, t')
        ec = np.ascontiguousarray(ec.transpose(3, 1, 4, 0, 2))  # (blk, p, t', m, b)
        in_maps.append({
            "a_mat": a_bf,
            "pi_vec": pi_in,
            "e_str": ec.reshape(NBLK, P, TBLK * W),
        })

    if _cached_nc is None:
        _cached_nc = _build_nc()
    res = run_bass_kernel_spmd(_cached_nc, in_maps, list(range(NCORES)))
    lls = np.concatenate([res.results[c]["out_ll"][0] for c in range(NCORES)])
    total = np.float64(lls.sum()) - np.float64(B) * T * np.log(np.float64(O))
    return np.asarray(np.float32(total))


# revision 14
# speedup vs baseline: 1.0770x; 1.0770x over previous
"""DiscreteHMM log-likelihood on 8 Trainium2 NeuronCores.

Math: the reference forward algorithm in log space,
    alpha_{t+1,j} = logsumexp_i(alpha_{t,i} + lA[i,j]) + lB[j, o_{t+1}]
is computed here in *probability* space (classic scaled forward algorithm):
    p_{t+1} = (p_t @ A) * E_{t+1},   A = softmax(log_A, rows), E_t = 1024*B[:, o_t]
The transition preserves total mass (A rows sum to 1) and the emission
multiply scales it by ~1/1024 on average (column means of a softmax row-
normalized 512x1024 table), so with the constant 1024 rescale folded into E
the running mass drifts only a few nats over all 512 steps (measured
[-4.1, +3.5] for these inputs) -- no per-step renormalization is needed.
Final per-sequence loglik = ln(sum_j p_T) - T*ln(1024).

Sharding: data-parallel over batch -- 8 sequences per core, parameters
replicated; per-sequence logliks are summed on host (64 adds).

Device layout (states-major): p is a (512 states x 8 batch) column block,
packed as ONE SBUF tile of (128, 32) bf16 -- column block m holds state
chunk j in [128m, 128m+128). Each step: 16 matmuls
psum[:, 8m:8m+8] += A[128k:,128m:].T @ p[:, 8k:8k+8] (A chunks stationary
128x128 bf16 weights, batch the 8-wide moving operand), then ONE DVE
multiply with the pre-gathered emission tile (128, 32) -> next p.
Emissions are gathered on host into a per-core stream with matching
(p, t, m, b) layout and double-buffered into SBUF in 64-step blocks.
"""

import numpy as np
import ml_dtypes
from contextlib import ExitStack

import concourse.bass as bass
import concourse.bacc as bacc
import concourse.mybir as mybir
import concourse.tile as tile
from concourse.bass_utils import run_bass_kernel_spmd

S = 512          # states
O = 1024         # observation symbols
B = 64           # batch
T = 512          # timesteps
NCORES = 8
BSH = B // NCORES          # sequences per core
P = 128                    # partition size
KC = S // P                # 4 state chunks
W = KC * BSH               # 32: packed free width of the p tile
TBLK = 64                  # timesteps per emission DMA block
NBLK = T // TBLK

F32 = mybir.dt.float32
BF16 = mybir.dt.bfloat16
_BF16_NP = ml_dtypes.bfloat16

_cached_nc = None


def _build_nc() -> bass.Bass:
    nc = bacc.Bacc()
    a_d = nc.dram_tensor("a_mat", (S, S), BF16, kind="ExternalInput")
    pi_d = nc.dram_tensor("pi_vec", (P, KC), F32, kind="ExternalInput")
    e_d = nc.dram_tensor("e_str", (NBLK, P, TBLK * W), F32, kind="ExternalInput")
    out_d = nc.dram_tensor("out_ll", (1, BSH), F32, kind="ExternalOutput")

    with ExitStack() as ctx:
        tc = ctx.enter_context(tile.TileContext(nc))
        const = ctx.enter_context(tc.tile_pool(name="const", bufs=1))
        epool = ctx.enter_context(tc.tile_pool(name="epool", bufs=2))
        ppool = ctx.enter_context(tc.tile_pool(name="ppool", bufs=5))
        pspool = ctx.enter_context(tc.tile_pool(name="psum", bufs=2, space="PSUM"))

        # prologue DMAs: one per A row-chunk (ordered by first use), one for
        # pi, and block-0 emissions as 4 quarter tiles so all transfers run
        # on parallel HWDGE queues.
        pi_t = const.tile([P, KC], F32, name="pi", tag="pi")
        nc.sync.dma_start(pi_t[:], pi_d[:, :])
        # block-0 emissions in uneven slices (first slice small so the scan
        # starts early); A chunks ordered by first use; late e slices issued
        # last so no early consumer shares a DMA-queue sem with them.
        E0SPLIT = (8, 8, 16, 32)
        E0OFF = (0, 8, 16, 32)
        e0q = []
        t_off = 0
        for i, n in enumerate(E0SPLIT):
            e0q.append(const.tile([P, n * W], F32, name=f"e0q{i}", tag=f"e0q{i}"))
        nc.sync.dma_start(e0q[0][:], e_d[0][:, 0:E0SPLIT[0] * W])
        a_t = {}
        for k in (2, 3, 0, 1):
            a_t[k] = const.tile([P, S], BF16, name=f"a{k}", tag=f"a{k}")
            nc.sync.dma_start(a_t[k][:], a_d[k * P:(k + 1) * P, :])
        for i in (1, 2, 3):
            nc.sync.dma_start(e0q[i][:],
                              e_d[0][:, E0OFF[i] * W:(E0OFF[i] + E0SPLIT[i]) * W])
        ones_t = const.tile([P, 1], BF16, name="ones", tag="ones")
        nc.vector.memset(ones_t[:], 1.0)

        def load_eblk(blk):
            et = epool.tile([P, TBLK * W], F32, name="eb", tag="eb")
            nc.sync.dma_start(et[:], e_d[blk])
            return et

        eb = None
        # p is held as two packed half tiles: pA = chunks {0,1}, pB = {2,3};
        # 3D (P, 2, BSH) so the DVE multiply covers both chunks in one op.
        pA = ppool.tile([P, 2, BSH], BF16, name="pA", tag="pA")
        pB = ppool.tile([P, 2, BSH], BF16, name="pB", tag="pB")
        for m in range(KC):
            dst = pA if m < 2 else pB
            nc.vector.tensor_scalar_mul(dst[:, m % 2, :],
                                        e0q[0][:, m * BSH:(m + 1) * BSH],
                                        pi_t[:, m:m + 1])

        def p_slice(k):
            src = pA if k < 2 else pB
            return src[:, k % 2, :]

        def e_slice(src_t, tt, half):
            ap = src_t[:, tt * W + half * 2 * BSH: tt * W + (half + 1) * 2 * BSH]
            return ap.rearrange("p (x b) -> p x b", b=BSH)

        # Matmul slot order + paired DVE multiplies chosen by simulating the
        # steady-state latency loop (MM drain -> sem -> DVE -> sem -> MM):
        # groups m2/m3 complete early and feed the first DVE op; their
        # chunks are consumed late in the next step. Accumulation groups
        # interleave, so each pair member gets its own PSUM bank: the pair
        # psum tile is (P, 2, 512) f32 = two banks, chunk m at [:, m%2, 0:8].
        SLOTS = [(2, 2), (0, 3), (3, 3), (1, 3), (2, 3), (3, 2), (3, 0), (2, 1),
                 (3, 1), (2, 0), (1, 2), (0, 1), (1, 0), (0, 2), (0, 0), (1, 1)]
        for blk in range(NBLK):
            if blk > 0:
                eb = load_eblk(blk)
            for tt in range(1 if blk == 0 else 0, TBLK):
                psA = pspool.tile([P, 2, 512], F32, name="psA", tag="psA")
                psB = pspool.tile([P, 2, 512], F32, name="psB", tag="psB")
                done = [0] * KC
                for (m, k) in SLOTS:
                    dst = psA if m < 2 else psB
                    done[m] += 1
                    nc.tensor.matmul(dst[:, m % 2, 0:BSH],
                                     a_t[k][:, m * P:(m + 1) * P], p_slice(k),
                                     start=(done[m] == 1), stop=(done[m] == KC),
                                     skip_group_check=True)
                if blk == 0:
                    qi = 0 if tt < 8 else (1 if tt < 16 else (2 if tt < 32 else 3))
                    esrc, ett = e0q[qi], tt - E0OFF[qi]
                else:
                    esrc, ett = eb, tt
                pB = ppool.tile([P, 2, BSH], BF16, name="pB", tag="pB")
                nc.vector.tensor_mul(pB[:], psB[:, :, 0:BSH], e_slice(esrc, ett, 1))
                pA = ppool.tile([P, 2, BSH], BF16, name="pA", tag="pA")
                nc.vector.tensor_mul(pA[:], psA[:, :, 0:BSH], e_slice(esrc, ett, 0))

        msum = pspool.tile([1, BSH], F32, name="msum", tag="psA")
        for k in range(KC):
            nc.tensor.matmul(msum[:], ones_t[:], p_slice(k),
                             start=(k == 0), stop=(k == KC - 1))
        lls = const.tile([1, BSH], F32, name="ll", tag="ll")
        nc.scalar.activation(lls[:], msum[:], mybir.ActivationFunctionType.Ln)
        nc.sync.dma_start(out_d[:, :], lls[:])
    nc.finalize()
    return nc


def _softmax(x, axis):
    x = x - x.max(axis=axis, keepdims=True)
    e = np.exp(x)
    return e / e.sum(axis=axis, keepdims=True)


def kernel(observations, log_pi, log_A, log_B):
    global _cached_nc
    obs = np.asarray(observations)
    A = _softmax(np.asarray(log_A, dtype=np.float64), 1)
    Bp = _softmax(np.asarray(log_B, dtype=np.float64), 1).astype(np.float32)
    pi = _softmax(np.asarray(log_pi, dtype=np.float64), 0).astype(np.float32)

    a_bf = A.astype(_BF16_NP)
    pi_in = np.ascontiguousarray(pi.reshape(KC, P).T)
    # X[j, b, t] = 1024 * B[j, o_{b,t}]
    X = (np.float32(O) * Bp[:, obs]).astype(np.float32)

    in_maps = []
    for c in range(NCORES):
        xc = X[:, c * BSH:(c + 1) * BSH, :]                    # (S, BSH, T)
        ec = xc.reshape(KC, P, BSH, NBLK, TBLK)                # (m, p, b, blk, t')
        ec = np.ascontiguousarray(ec.transpose(3, 1, 4, 0, 2))  # (blk, p, t', m, b)
        in_maps.append({
            "a_mat": a_bf,
            "pi_vec": pi_in,
            "e_str": ec.reshape(NBLK, P, TBLK * W),
        })

    if _cached_nc is None:
        _cached_nc = _build_nc()
    res = run_bass_kernel_spmd(_cached_nc, in_maps, list(range(NCORES)))
    lls = np.concatenate([res.results[c]["out_ll"][0] for c in range(NCORES)])
    total = np.float64(lls.sum()) - np.float64(B) * T * np.log(np.float64(O))
    return np.asarray(np.float32(total))
